# revision 32
# baseline (speedup 1.0000x reference)
"""Trainium2 Bass kernel for the CrossAttention reference module (v2).

  claim = x[claim_index]; evidence = x[evidence_index]
  wc = claim @ Wc + bc; we = evidence @ We + be
  S = wc @ we.T + blockdiag_mask(batch[claim_index], batch[evidence_index])
  A = softmax(S, -1); cn = A @ evidence
  a = concat([claim, cn, claim-cn, claim*cn]) @ Wa + ba
  out = segment_mean(a, batch[claim_index], 64)

Sharding: claims sorted by graph, 512 per core across 8 cores; evidence
sorted by graph so each core's evidence is a contiguous slice (<= 1280
rows).  The host pre-gathers rows, computes the 64-dim projections in
fp32 (wc/we), folds Wa (W1'=Wa0+Wa2, W2'=Wa1-Wa2, W3'=Wa3), and builds
the one-hot mask rows; the device does the O(Nc*Ne) attention work.

Device math per core (all matmuls bf16 with fp32 PSUM):
  S^T[e-tile] = we_aug[:,e128]^T . wc_aug[:, win(e)]   (windowed: only the
      claim tiles whose graphs can intersect this evidence tile)
  P^T = exp(S^T - 1034)   (one-hot rows add +1024 on same-graph pairs; the
      -10 shift keeps P in normal bf16/f32 range; row-constant so softmax
      ignores it; cross-graph pairs underflow to exactly 0)
  block-sparse flipped PV: cn[c-tile t] += P^T[:,e,t128]^T . ev[e]  only for
      e in span(t)  -> cn lands c-major [128c, 512h], no transposes
  rowsum[c] via 1-column matmuls sharing the PV stationary (near-free)
  cn = pv * (1/rowsum)  (DVE reciprocal on [128,1] + tensor_scalar)
  pool-then-project:  segX^T[k128, 64g] = block[c,k]^T . onehot(graph)
      for the 12 k-tiles of [claim | cn | claim*cn]  (segment-sum BEFORE the
      1536-wide output matmul -- 512 claims pool to 64 graphs first)
  seg[64, 512] = sum_j segX^T[j]^T . Wa'[j]
  host divides by counts and adds ba (mean(a+ba)=mean(a)+ba)
"""

import os
import sys

if "/opt/trn_rl_repo" not in sys.path:
    sys.path.insert(0, "/opt/trn_rl_repo")

import ml_dtypes
import numpy as np

import concourse.bass as bass
import concourse.mybir as mybir
import concourse.tile as tile
from concourse.bass_utils import run_bass_kernel_spmd
from concourse.vector_clock import ScopedClock

P = 128
NHID = 512
PROJ = 64
NC_ALL = 4096
NE = 8192
NG = 64
N_CORES = 8
NC_LOC = NC_ALL // N_CORES  # 512 claims per core
CT = NC_LOC // P            # 4 claim tiles per core
KO = 12                     # folded concat k-tiles (3 blocks x 4)
MAG = 32.0                  # sqrt(1024): one-hot scale
EXP_BIAS = -(MAG * MAG + 10.0)  # exp(S + 1024 - 1034) = exp(S - 10)

f32 = mybir.dt.float32
bf16 = mybir.dt.bfloat16
AF = mybir.ActivationFunctionType
ALU = mybir.AluOpType
nbf16 = ml_dtypes.bfloat16


class _PatchedTileContext(tile.TileContext):
    """Workaround: this neuronxcc/walrus build rejects InstDrain carrying
    sync waits ("Too many sync wait commands").  Collect the final drain's
    waits on nops (one wait each) and emit the drain itself wait-free.
    Also slimmed teardown: one barrier instead of two."""

    def _drain_and_barrier(self, tick_clock, wait_clock):
        nc = self.nc
        nop0 = nc.sync.nop(nofuse=True)
        wait_clock.add_sem_waits(nop0.ins, ScopedClock({None: tick_clock.global_clock}))
        si = nop0.ins.sync_info
        waits = list(si.on_wait) if si and si.on_wait else []
        if si and len(waits) > 1:
            del si.on_wait[1:]
            for w in waits[1:]:
                extra = nc.sync.nop(nofuse=True)
                if extra.ins.sync_info is None:
                    extra.ins.sync_info = mybir.SyncInfo(on_wait=[w], on_update=[])
                else:
                    extra.ins.sync_info.on_wait.append(w)
        drain_inst = nc.sync.drain()
        wait_clock.add_sem_waits(
            drain_inst.ins, ScopedClock({None: tick_clock.global_clock})
        )
        dsi = drain_inst.ins.sync_info
        if dsi and dsi.on_wait:
            del dsi.on_wait[:]
        nc.all_engine_barrier()
        popped = nc._tile_sem_poison_stack.pop()
        assert popped is self._sem_poison
        nc.clear_and_free_semaphores(list(self.sems.allocated().values()))


def _split_excess_waits(nc: bass.Bass, limit: int = 1) -> None:
    """This walrus build rejects instructions carrying more than ~1 sync
    wait.  Move excess waits onto injected same-engine nops (engines are
    in-order, so gating a preceding nop gates the instruction)."""
    for f in nc.m.functions:
        for bb in f.blocks:
            new_insts = []
            for inst in bb.instructions:
                si = getattr(inst, "sync_info", None)
                if si is not None and si.on_wait and len(si.on_wait) > limit:
                    keep = list(si.on_wait[-limit:])
                    excess = list(si.on_wait[:-limit])
                    for w in excess:
                        nop = mybir.InstNoOp(
                            name=f"I-{nc.next_id()}", engine=inst.engine,
                            ins=[], outs=[],
                            sync_info=mybir.SyncInfo(on_wait=[w], on_update=[]))
                        new_insts.append(nop)
                    del si.on_wait[:]
                    si.on_wait.extend(keep)
                new_insts.append(inst)
            bb.instructions[:] = new_insts


def build_nc(struct: dict, reps: int = 1, split_waits: bool = True) -> bass.Bass:
    ne_loc = struct["ne_loc"]
    ET = struct["et"]
    spans = struct["spans"]      # per c-tile t: (lo, hi) e-tile range
    windows = struct["windows"]  # per e-tile: (c0, c1) claim-col window
    assert ne_loc == ET * P
    KD = 8                       # device output k-tiles (cn, ml blocks only)

    nc = bass.Bass("TRN2", target_bir_lowering=False, debug=False,
                   num_devices=N_CORES)

    # All bf16 inputs live in ONE dram blob mirroring one big SBUF tile, so a
    # handful of column-range DMAs move everything (each dma_start costs
    # ~650ns of serial issue time on the sync queue -- fewer is faster).
    # layout: [wca | wea | ev(0..ET) | cl | ohs | wab(8)]
    o_wca = 0
    o_wea = o_wca + NC_LOC
    o_ev = o_wea + ne_loc
    o_cl = o_ev + ET * NHID
    o_ohs = o_cl + CT * NHID
    o_wab = o_ohs + CT * NG
    TOT = o_wab + KD * NHID
    struct["mega_cols"] = TOT

    mega_d = nc.dram_tensor("mega", [P, TOT], bf16, kind="ExternalInput").ap()
    seg_d = nc.dram_tensor("seg", [NG, NHID], f32, kind="ExternalOutput").ap()

    N_WARM = int(os.environ.get("KWARM", "5"))  # PE p-state warm-up matmuls

    with _PatchedTileContext(nc) as tc:
        with (
            tc.tile_pool(name="const", bufs=1) as cpool,
            # PSUM banks: scores ring 2 + pv 4 + rs 1 + bankA 1 = 8
            tc.tile_pool(name="psS", bufs=2, space="PSUM") as psS,
            tc.tile_pool(name="psV", bufs=1, space="PSUM") as psV,
        ):
            # ---------- constants ----------
            scratch = cpool.tile([P, P], bf16)
            nc.vector.memset(scratch[:], 0.5)
            exp_bias = cpool.tile([P, 1], f32)
            nc.gpsimd.memset(exp_bias[:], EXP_BIAS)
            ones_col = cpool.tile([P, 1], bf16)
            nc.gpsimd.memset(ones_col[:], 1.0)

            mega = cpool.tile([P, TOT], bf16)
            wca_sb = mega[:, o_wca:o_wca + NC_LOC]
            wea_sb = mega[:, o_wea:o_wea + ne_loc]

            def ev_s(e, a=0, b=NHID):
                return mega[:, o_ev + e * NHID + a:o_ev + e * NHID + b]

            def cl_s(t, a=0, b=NHID):
                return mega[:, o_cl + t * NHID + a:o_cl + t * NHID + b]

            def ohs_s(t):
                return mega[:, o_ohs + t * NG:o_ohs + (t + 1) * NG]

            def wab_s(j):
                return mega[:, o_wab + j * NHID:o_wab + (j + 1) * NHID]

            # single-queue DMA in need order: the HBM path sustains ~360GB/s
            # once flowing but the first descriptor has ~2us of dead time, so
            # a SMALL first chunk + strict priority order beats parallel
            # queues (which contend for the same bandwidth)
            chunks = [
                (o_wca, o_wea + 2 * P),              # wca + wea[:,0:256]
                (o_ev, o_ev + NHID),                 # ev[0]
                (o_wea + 2 * P, o_wea + 6 * P),      # wea[:,256:768]
                (o_ev + NHID, o_ev + 2 * NHID),      # ev[1]
                (o_wea + 6 * P, o_ev),               # wea rest
                (o_ev + 2 * NHID, o_ev + 3 * NHID),  # ev[2]
                (o_ev + 3 * NHID, o_ev + 5 * NHID),  # ev[3:5]
                (o_cl, o_wab),                       # cl + ohs
                (o_ev + 5 * NHID, o_cl),             # ev[5:ET]
                (o_wab, TOT),                        # wab
            ]
            for a, b in chunks:
                if b > a:
                    nc.sync.dma_start(mega[:, a:b], mega_d[:, a:b])

            # ---------- PE p-state warm-up during DMA wait ----------
            for i in range(N_WARM):
                warm_ps = psS.tile([P, NHID], f32, tag="s")
                nc.tensor.matmul(warm_ps[:, :P], scratch[:], scratch[:],
                                 start=True, stop=True)

            with tc.tile_pool(name="work", bufs=2) as wpool:
                for rep in range(reps):
                    p_sb = wpool.tile([P, ET, NC_LOC], bf16, tag="p")
                    pvs = [psV.tile([P, NHID], f32, tag=f"pv{t}",
                                    name=f"pv{t}_{rep}") for t in range(CT)]
                    rs = psV.tile([P, NHID], f32, tag="rs", name=f"rs_{rep}")
                    bankA = psV.tile([P, NHID], f32, tag="bankA",
                                     name=f"bankA_{rep}")
                    rcp = wpool.tile([P, CT], f32, tag="rcp")
                    cn_sb = wpool.tile([P, CT, NHID], bf16, tag="cn")
                    ml_sb = wpool.tile([P, CT, NHID], bf16, tag="ml")
                    segXT = wpool.tile([P, KD, PROJ], bf16, tag="sxt")
                    seg_sb = wpool.tile([NG, NHID], f32, tag="segsb")

                    pend = []          # deferred pool matmuls
                    first_rs = [True]  # first write into the rs bank
                    first_A = [True]   # first write into bankA

                    def pop_pools(n):
                        for _ in range(min(n, len(pend))):
                            pend.pop(0)()

                    def queue_pool(j, t, blk, h):
                        def mk():
                            nc.tensor.matmul(
                                bankA[:, j * PROJ:(j + 1) * PROJ],
                                blk[:, t, h * P:(h + 1) * P],
                                ohs_s(t), start=first_A[0],
                                stop=(t == CT - 1),
                                skip_group_check=True)
                            first_A[0] = False
                        pend.append(mk)

                    def emit_complete(t):
                        # normalize + claim*cn products, pipelined in 128-col
                        # chunks across DVE/ACT/GpSimd so the tail chain for
                        # the last tile is short
                        lo, hi = spans[t]
                        use_act = (hi == ET)  # ACT is done with exps by then
                        nc.vector.reciprocal(rcp[:, t:t + 1], rs[:, t:t + 1])
                        for h in range(CT):
                            a, b = h * P, (h + 1) * P
                            if use_act and h % 2 == 1:
                                nc.scalar.activation(cn_sb[:, t, a:b],
                                                     pvs[t][:, a:b],
                                                     AF.Identity,
                                                     scale=rcp[:, t:t + 1])
                            else:
                                nc.vector.tensor_scalar_mul(
                                    cn_sb[:, t, a:b], pvs[t][:, a:b],
                                    rcp[:, t:t + 1])
                            eng = nc.gpsimd if h % 2 == 0 else nc.vector
                            eng.tensor_tensor(
                                out=ml_sb[:, t, a:b], in0=cl_s(t, a, b),
                                in1=cn_sb[:, t, a:b], op=ALU.mult)
                            queue_pool(h, t, cn_sb, h)
                            queue_pool(4 + h, t, ml_sb, h)

                    def emit_pv_batch(e):
                        for t in range(CT):
                            lo, hi = spans[t]
                            if not (lo <= e < hi):
                                continue
                            nc.tensor.matmul(pvs[t][:],
                                             p_sb[:, e, t * P:(t + 1) * P],
                                             ev_s(e),
                                             start=(e == lo), stop=(e == hi - 1))
                            pop_pools(1)
                            nc.tensor.matmul(rs[:, t:t + 1],
                                             p_sb[:, e, t * P:(t + 1) * P],
                                             ones_col[:], start=first_rs[0],
                                             stop=(e == hi - 1),
                                             skip_group_check=True)
                            first_rs[0] = False
                            pop_pools(1)
                        for t in range(CT):
                            lo, hi = spans[t]
                            if e == hi - 1:
                                emit_complete(t)

                    # ---------- scores -> exp -> PV e-loop (sw-pipelined) ----
                    # exp is emitted per claim-tile slice so each PV matmul
                    # only waits for its own 128-col chunk, not the window
                    e_prev = None
                    for e in range(ET):
                        w0, w1 = windows[e]
                        s_ps = psS.tile([P, NHID], f32, tag="s",
                                        name=f"s{e}_{rep}")
                        nc.tensor.matmul(s_ps[:, :w1 - w0],
                                         wea_sb[:, e * P:(e + 1) * P],
                                         wca_sb[:, w0:w1], start=True, stop=True)
                        for t in range(CT):
                            lo, hi = spans[t]
                            if lo <= e < hi:
                                a = t * P - w0
                                nc.scalar.activation(
                                    p_sb[:, e, t * P:(t + 1) * P],
                                    s_ps[:, a:a + P], AF.Exp,
                                    bias=exp_bias[:])
                        if e_prev is not None:
                            emit_pv_batch(e_prev)
                        e_prev = e
                    emit_pv_batch(e_prev)

                    # ---------- tail: drain pools, copy segX^T, final matmul --
                    fin = psS.tile([P, NHID], f32, tag="s", name=f"fin_{rep}")
                    pop_pools(len(pend))
                    for j in range(KD):
                        nc.scalar.copy(segXT[:, j, :],
                                       bankA[:, j * PROJ:(j + 1) * PROJ])
                        nc.tensor.matmul(fin[:NG, :], segXT[:, j, :],
                                         wab_s(j), start=(j == 0),
                                         stop=(j == KD - 1))

                    nc.scalar.copy(seg_sb[:], fin[:NG, :])
                    nc.sync.dma_start(seg_d[:], seg_sb[:])
    if split_waits:
        _split_excess_waits(nc)
    return nc


def make_in_maps(inputs: dict):
    """Host-side sharding: sort claims+evidence by graph, fp32 projections,
    pre-gather x rows (bf16) into per-core SBUF layouts, and compute the
    block-sparse envelope structure shared by all cores (SPMD)."""
    batch = np.asarray(inputs["batch"]).astype(np.int64)
    ci = np.asarray(inputs["claim_index"]).astype(np.int64)
    ei = np.asarray(inputs["evidence_index"]).astype(np.int64)
    x = np.asarray(inputs["x"], dtype=np.float32)
    cb = batch[ci]
    eb = batch[ei]
    counts = np.bincount(cb, minlength=NG).astype(np.float32)
    ba = np.asarray(inputs["ba"], dtype=np.float32).reshape(NHID)

    order_c = np.argsort(cb, kind="stable")
    cb_s = cb[order_c]
    order_e = np.argsort(eb, kind="stable")
    eb_s = eb[order_e]

    xc = x[ci[order_c]]             # [4096, 512] f32 sorted claims
    xe = x[ei[order_e]]             # [8192, 512] f32 sorted evidence
    ev_starts = np.searchsorted(eb_s, np.arange(NG + 1))

    # per-core contiguous evidence spans
    raw_spans = []
    for c in range(N_CORES):
        g_lo = int(cb_s[c * NC_LOC])
        g_hi = int(cb_s[(c + 1) * NC_LOC - 1])
        lo, hi = int(ev_starts[g_lo]), int(ev_starts[g_hi + 1])
        raw_spans.append((lo, hi))
    ne_loc = max(512, -(-max(hi - lo for lo, hi in raw_spans) // P) * P)
    ne_loc = min(ne_loc, NE)
    ET = ne_loc // P

    Wc = np.asarray(inputs["Wc"], dtype=np.float32)
    We = np.asarray(inputs["We"], dtype=np.float32)
    bc = np.asarray(inputs["bc"], dtype=np.float32).reshape(PROJ)
    be = np.asarray(inputs["be"], dtype=np.float32).reshape(PROJ)
    Wa = np.asarray(inputs["Wa"], dtype=np.float32)
    W1 = Wa[0:NHID] + Wa[2 * NHID:3 * NHID]
    W2 = Wa[NHID:2 * NHID] - Wa[2 * NHID:3 * NHID]
    W3 = Wa[3 * NHID:4 * NHID]
    # device handles the attention-dependent blocks (cn, ml); the claim
    # block's pooled contribution (Oh^T cl) @ W1' is computed on host
    wab = np.concatenate([W2, W3], axis=0).astype(nbf16)  # [1024, 512]

    # choose each core's evidence-window start (within its slack) to minimize
    # the cross-core envelope of per-claim-tile e-tile spans (coordinate
    # descent; spans quantize at 128-row tiles so candidates step by 32)
    def tile_spans(c, start):
        cb_c = cb_s[c * NC_LOC:(c + 1) * NC_LOC]
        out = []
        for t in range(CT):
            gmin = int(cb_c[t * P])
            gmax = int(cb_c[(t + 1) * P - 1])
            r0 = int(ev_starts[gmin]) - start
            r1 = int(ev_starts[gmax + 1]) - start
            out.append((r0 // P, -(-r1 // P)))
        return out

    # window start may be anywhere in [hi - ne_loc, lo]; rows past the end of
    # the global evidence array are padded with a sentinel graph (masked out),
    # so cores near the end are not force-slid into earlier graphs
    start_rng = []
    for c in range(N_CORES):
        lo, hi = raw_spans[c]
        start_rng.append((max(0, hi - ne_loc), lo))
    starts = [s_hi for (s_lo, s_hi) in start_rng]

    def env_cost(st):
        allsp = [tile_spans(c, st[c]) for c in range(N_CORES)]
        cost = 0
        for t in range(CT):
            cost += max(s[t][1] for s in allsp) - min(s[t][0] for s in allsp)
        return cost

    for _ in range(3):
        for c in range(N_CORES):
            s_lo, s_hi = start_rng[c]
            best = (env_cost(starts), starts[c])
            for cand in range(s_lo, s_hi + 1, 32):
                starts[c] = cand
                cc = env_cost(starts)
                if cc < best[0]:
                    best = (cc, cand)
            starts[c] = best[1]

    g_ids = np.arange(NG)
    in_maps = []
    env_spans = [[ET, 0] for _ in range(CT)]
    host_cl = np.zeros((NG, NHID), np.float64)
    for c in range(N_CORES):
        lo = starts[c]
        end = min(lo + ne_loc, NE)
        pad = lo + ne_loc - end
        xe_c = xe[lo:end]                          # [<=ne_loc, 512] f32
        eb_c = eb_s[lo:end]
        if pad:
            xe_c = np.concatenate([xe_c, np.zeros((pad, NHID), np.float32)], 0)
            eb_c = np.concatenate([eb_c, np.full(pad, 9999, eb_c.dtype)], 0)
        xc_c = xc[c * NC_LOC:(c + 1) * NC_LOC]     # [512, 512] f32
        cb_c = cb_s[c * NC_LOC:(c + 1) * NC_LOC]

        wc = (xc_c @ Wc + bc).T                    # [64, 512] f32
        we = (xe_c @ We + be).T                    # [64, ne_loc] f32
        oh = (cb_c[:, None] == g_ids[None, :])     # [512, 64]
        host_cl += (oh.T.astype(np.float64) @ xc_c) @ W1

        wca = np.concatenate(
            [wc, MAG * (cb_c[None, :] == g_ids[:PROJ, None])], 0).astype(nbf16)
        wea = np.concatenate(
            [we, MAG * (eb_c[None, :] == g_ids[:PROJ, None])], 0).astype(nbf16)
        ev = xe_c.astype(nbf16).reshape(ET, P, NHID).transpose(1, 0, 2)
        cl = xc_c.astype(nbf16).reshape(CT, P, NHID).transpose(1, 0, 2)
        ohs = oh.reshape(CT, P, NG).transpose(1, 0, 2).astype(nbf16)
        wabt = wab.reshape(8, P, NHID).transpose(1, 0, 2)
        mega = np.concatenate(
            [wca, wea,
             ev.reshape(P, -1), cl.reshape(P, -1), ohs.reshape(P, -1),
             wabt.reshape(P, -1)], axis=1)
        in_maps.append({"mega": np.ascontiguousarray(mega)})

        # per-core per-c-tile evidence e-tile spans -> envelope
        for t in range(CT):
            gmin = int(cb_c[t * P])
            gmax = int(cb_c[(t + 1) * P - 1])
            r0 = int(np.searchsorted(eb_c, gmin))
            r1 = int(np.searchsorted(eb_c, gmax, side="right"))
            assert r1 > r0, "claim tile with no evidence in its graphs"
            env_spans[t][0] = min(env_spans[t][0], r0 // P)
            env_spans[t][1] = max(env_spans[t][1], -(-r1 // P))

    # enforce monotone lo/hi (expand-only) so claim windows are contiguous
    for t in range(CT - 2, -1, -1):
        env_spans[t][0] = min(env_spans[t][0], env_spans[t + 1][0])
    for t in range(1, CT):
        env_spans[t][1] = max(env_spans[t][1], env_spans[t - 1][1])
    spans = [(lo, hi) for lo, hi in env_spans]

    windows = []
    for e in range(ET):
        ts = [t for t in range(CT) if spans[t][0] <= e < spans[t][1]]
        assert ts, f"e-tile {e} covered by no claim tile"
        assert ts == list(range(min(ts), max(ts) + 1))
        windows.append((min(ts) * P, (max(ts) + 1) * P))

    struct = {"ne_loc": ne_loc, "et": ET, "spans": spans, "windows": windows,
              "host_cl": host_cl}
    return in_maps, counts, ba, struct


def postprocess(results: list, counts: np.ndarray, ba: np.ndarray,
                struct: dict) -> np.ndarray:
    seg = struct["host_cl"].copy()
    for c in range(N_CORES):
        seg += results[c]["seg"].astype(np.float64)
    # segment_mean(a + ba) = segment_mean(a) + ba, except empty graphs stay 0
    out = seg / np.maximum(counts, 1.0)[:, None] + (counts > 0)[:, None] * ba[None, :]
    return out.astype(np.float32)


def kernel(**inputs) -> np.ndarray:
    in_maps, counts, ba, struct = make_in_maps(inputs)
    nc = build_nc(struct)
    res = run_bass_kernel_spmd(nc, in_maps, list(range(N_CORES)))
    return postprocess(res.results, counts, ba, struct)


# revision 34
# speedup vs baseline: 1.0316x; 1.0316x over previous
"""Trainium2 Bass kernel for the CrossAttention reference module (v2).

  claim = x[claim_index]; evidence = x[evidence_index]
  wc = claim @ Wc + bc; we = evidence @ We + be
  S = wc @ we.T + blockdiag_mask(batch[claim_index], batch[evidence_index])
  A = softmax(S, -1); cn = A @ evidence
  a = concat([claim, cn, claim-cn, claim*cn]) @ Wa + ba
  out = segment_mean(a, batch[claim_index], 64)

Sharding: claims sorted by graph, 512 per core across 8 cores; evidence
sorted by graph so each core's evidence is a contiguous slice (<= 1280
rows).  The host pre-gathers rows, computes the 64-dim projections in
fp32 (wc/we), folds Wa (W1'=Wa0+Wa2, W2'=Wa1-Wa2, W3'=Wa3), and builds
the one-hot mask rows; the device does the O(Nc*Ne) attention work.

Device math per core (all matmuls bf16 with fp32 PSUM):
  S^T[e-tile] = we_aug[:,e128]^T . wc_aug[:, win(e)]   (windowed: only the
      claim tiles whose graphs can intersect this evidence tile)
  P^T = exp(S^T - 1034)   (one-hot rows add +1024 on same-graph pairs; the
      -10 shift keeps P in normal bf16/f32 range; row-constant so softmax
      ignores it; cross-graph pairs underflow to exactly 0)
  block-sparse flipped PV: cn[c-tile t] += P^T[:,e,t128]^T . ev[e]  only for
      e in span(t)  -> cn lands c-major [128c, 512h], no transposes
  rowsum[c] via 1-column matmuls sharing the PV stationary (near-free)
  cn = pv * (1/rowsum)  (DVE reciprocal on [128,1] + tensor_scalar)
  pool-then-project:  segX^T[k128, 64g] = block[c,k]^T . onehot(graph)
      for the 12 k-tiles of [claim | cn | claim*cn]  (segment-sum BEFORE the
      1536-wide output matmul -- 512 claims pool to 64 graphs first)
  seg[64, 512] = sum_j segX^T[j]^T . Wa'[j]
  host divides by counts and adds ba (mean(a+ba)=mean(a)+ba)
"""

import os
import sys

if "/opt/trn_rl_repo" not in sys.path:
    sys.path.insert(0, "/opt/trn_rl_repo")

import ml_dtypes
import numpy as np

import concourse.bass as bass
import concourse.mybir as mybir
import concourse.tile as tile
from concourse.bass_utils import run_bass_kernel_spmd
from concourse.vector_clock import ScopedClock

P = 128
NHID = 512
PROJ = 64
NC_ALL = 4096
NE = 8192
NG = 64
N_CORES = 8
NC_LOC = NC_ALL // N_CORES  # 512 claims per core
CT = NC_LOC // P            # 4 claim tiles per core
KO = 12                     # folded concat k-tiles (3 blocks x 4)
MAG = 32.0                  # sqrt(1024): one-hot scale
EXP_BIAS = -(MAG * MAG + 10.0)  # exp(S + 1024 - 1034) = exp(S - 10)

f32 = mybir.dt.float32
bf16 = mybir.dt.bfloat16
AF = mybir.ActivationFunctionType
ALU = mybir.AluOpType
nbf16 = ml_dtypes.bfloat16


class _PatchedTileContext(tile.TileContext):
    """Workaround: this neuronxcc/walrus build rejects InstDrain carrying
    sync waits ("Too many sync wait commands").  Collect the final drain's
    waits on nops (one wait each) and emit the drain itself wait-free.
    Also slimmed teardown: one barrier instead of two."""

    def _drain_and_barrier(self, tick_clock, wait_clock):
        nc = self.nc
        nop0 = nc.sync.nop(nofuse=True)
        wait_clock.add_sem_waits(nop0.ins, ScopedClock({None: tick_clock.global_clock}))
        si = nop0.ins.sync_info
        waits = list(si.on_wait) if si and si.on_wait else []
        if si and len(waits) > 1:
            del si.on_wait[1:]
            for w in waits[1:]:
                extra = nc.sync.nop(nofuse=True)
                if extra.ins.sync_info is None:
                    extra.ins.sync_info = mybir.SyncInfo(on_wait=[w], on_update=[])
                else:
                    extra.ins.sync_info.on_wait.append(w)
        drain_inst = nc.sync.drain()
        wait_clock.add_sem_waits(
            drain_inst.ins, ScopedClock({None: tick_clock.global_clock})
        )
        dsi = drain_inst.ins.sync_info
        if dsi and dsi.on_wait:
            del dsi.on_wait[:]
        nc.all_engine_barrier()
        popped = nc._tile_sem_poison_stack.pop()
        assert popped is self._sem_poison
        nc.clear_and_free_semaphores(list(self.sems.allocated().values()))


def _split_excess_waits(nc: bass.Bass, limit: int = 1) -> None:
    """This walrus build rejects instructions carrying more than ~1 sync
    wait.  Move excess waits onto injected same-engine nops (engines are
    in-order, so gating a preceding nop gates the instruction)."""
    for f in nc.m.functions:
        for bb in f.blocks:
            new_insts = []
            for inst in bb.instructions:
                si = getattr(inst, "sync_info", None)
                if si is not None and si.on_wait and len(si.on_wait) > limit:
                    keep = list(si.on_wait[-limit:])
                    excess = list(si.on_wait[:-limit])
                    for w in excess:
                        nop = mybir.InstNoOp(
                            name=f"I-{nc.next_id()}", engine=inst.engine,
                            ins=[], outs=[],
                            sync_info=mybir.SyncInfo(on_wait=[w], on_update=[]))
                        new_insts.append(nop)
                    del si.on_wait[:]
                    si.on_wait.extend(keep)
                new_insts.append(inst)
            bb.instructions[:] = new_insts


def build_nc(struct: dict, reps: int = 1, split_waits: bool = True) -> bass.Bass:
    ne_loc = struct["ne_loc"]
    ET = struct["et"]
    spans = struct["spans"]      # per c-tile t: (lo, hi) e-tile range
    windows = struct["windows"]  # per e-tile: (c0, c1) claim-col window
    assert ne_loc == ET * P
    KD = 8                       # device output k-tiles (cn, ml blocks only)

    nc = bass.Bass("TRN2", target_bir_lowering=False, debug=False,
                   num_devices=N_CORES)

    # All bf16 inputs live in ONE dram blob mirroring one big SBUF tile, so a
    # handful of column-range DMAs move everything (each dma_start costs
    # ~650ns of serial issue time on the sync queue -- fewer is faster).
    # layout: [wca | wea | ev(0..ET) | cl | ohs | wab(8)]
    o_wca = 0
    o_wea = o_wca + NC_LOC
    o_ev = o_wea + ne_loc
    o_cl = o_ev + ET * NHID
    o_ohs = o_cl + CT * NHID
    o_wab = o_ohs + CT * NG
    TOT = o_wab + KD * NHID
    struct["mega_cols"] = TOT

    mega_d = nc.dram_tensor("mega", [P, TOT], bf16, kind="ExternalInput").ap()
    seg_d = nc.dram_tensor("seg", [NG, NHID], f32, kind="ExternalOutput").ap()

    N_WARM = int(os.environ.get("KWARM", "5"))  # PE p-state warm-up matmuls

    with _PatchedTileContext(nc) as tc:
        with (
            tc.tile_pool(name="const", bufs=1) as cpool,
            # PSUM banks: scores ring 2 + pv 4 + rs 1 + bankA 1 = 8
            tc.tile_pool(name="psS", bufs=2, space="PSUM") as psS,
            tc.tile_pool(name="psV", bufs=1, space="PSUM") as psV,
        ):
            # ---------- constants ----------
            scratch = cpool.tile([P, P], bf16)
            nc.vector.memset(scratch[:], 0.5)
            exp_bias = cpool.tile([P, 1], f32)
            nc.gpsimd.memset(exp_bias[:], EXP_BIAS)
            ones_col = cpool.tile([P, 1], bf16)
            nc.gpsimd.memset(ones_col[:], 1.0)

            mega = cpool.tile([P, TOT], bf16)
            wca_sb = mega[:, o_wca:o_wca + NC_LOC]
            wea_sb = mega[:, o_wea:o_wea + ne_loc]

            def ev_s(e, a=0, b=NHID):
                return mega[:, o_ev + e * NHID + a:o_ev + e * NHID + b]

            def cl_s(t, a=0, b=NHID):
                return mega[:, o_cl + t * NHID + a:o_cl + t * NHID + b]

            def ohs_s(t):
                return mega[:, o_ohs + t * NG:o_ohs + (t + 1) * NG]

            def wab_s(j):
                return mega[:, o_wab + j * NHID:o_wab + (j + 1) * NHID]

            # single-queue DMA in need order: the HBM path sustains ~360GB/s
            # once flowing but the first descriptor has ~2us of dead time, so
            # a SMALL first chunk + strict priority order beats parallel
            # queues (which contend for the same bandwidth)
            chunks = [
                (o_wca, o_wea + 2 * P),              # wca + wea[:,0:256]
                (o_ev, o_ev + NHID),                 # ev[0]
                (o_wea + 2 * P, o_wea + 6 * P),      # wea[:,256:768]
                (o_ev + NHID, o_ev + 2 * NHID),      # ev[1]
                (o_wea + 6 * P, o_ev),               # wea rest
                (o_ev + 2 * NHID, o_ev + 3 * NHID),  # ev[2]
                (o_ev + 3 * NHID, o_ev + 5 * NHID),  # ev[3:5]
                (o_cl, o_wab),                       # cl + ohs
                (o_ev + 5 * NHID, o_cl),             # ev[5:ET]
                (o_wab, TOT),                        # wab
            ]
            for a, b in chunks:
                if b > a:
                    nc.sync.dma_start(mega[:, a:b], mega_d[:, a:b])

            # ---------- PE p-state warm-up during DMA wait ----------
            for i in range(N_WARM):
                warm_ps = psS.tile([P, NHID], f32, tag="s")
                nc.tensor.matmul(warm_ps[:, :P], scratch[:], scratch[:],
                                 start=True, stop=True)

            with tc.tile_pool(name="work", bufs=2) as wpool:
                for rep in range(reps):
                    p_sb = wpool.tile([P, ET, NC_LOC], bf16, tag="p")
                    pvs = [psV.tile([P, NHID], f32, tag=f"pv{t}",
                                    name=f"pv{t}_{rep}") for t in range(CT)]
                    rs = psV.tile([P, NHID], f32, tag="rs", name=f"rs_{rep}")
                    bankA = psV.tile([P, NHID], f32, tag="bankA",
                                     name=f"bankA_{rep}")
                    rcp = wpool.tile([P, CT], f32, tag="rcp")
                    cn_sb = wpool.tile([P, CT, NHID], bf16, tag="cn")
                    ml_sb = wpool.tile([P, CT, NHID], bf16, tag="ml")
                    segXT = wpool.tile([P, KD, PROJ], bf16, tag="sxt")
                    seg_sb = wpool.tile([NG, NHID], f32, tag="segsb")

                    pend = []          # deferred pool matmuls
                    first_rs = [True]  # first write into the rs bank
                    first_A = [True]   # first write into bankA

                    def pop_pools(n):
                        for _ in range(min(n, len(pend))):
                            pend.pop(0)()

                    def queue_pool(j, t, blk, h):
                        def mk():
                            nc.tensor.matmul(
                                bankA[:, j * PROJ:(j + 1) * PROJ],
                                blk[:, t, h * P:(h + 1) * P],
                                ohs_s(t), start=first_A[0],
                                stop=(t == CT - 1),
                                skip_group_check=True)
                            first_A[0] = False
                        pend.append(mk)

                    def emit_complete(t):
                        # normalize + claim*cn products, pipelined in 128-col
                        # chunks across DVE/ACT/GpSimd so the tail chain for
                        # the last tile is short
                        lo, hi = spans[t]
                        use_act = (hi == ET)  # ACT is done with exps by then
                        nc.vector.reciprocal(rcp[:, t:t + 1], rs[:, t:t + 1])
                        for h in range(CT):
                            a, b = h * P, (h + 1) * P
                            if use_act and h % 2 == 1:
                                nc.scalar.activation(cn_sb[:, t, a:b],
                                                     pvs[t][:, a:b],
                                                     AF.Identity,
                                                     scale=rcp[:, t:t + 1])
                            else:
                                nc.vector.tensor_scalar_mul(
                                    cn_sb[:, t, a:b], pvs[t][:, a:b],
                                    rcp[:, t:t + 1])
                            eng = nc.gpsimd if h % 2 == 0 else nc.vector
                            eng.tensor_tensor(
                                out=ml_sb[:, t, a:b], in0=cl_s(t, a, b),
                                in1=cn_sb[:, t, a:b], op=ALU.mult)
                            queue_pool(h, t, cn_sb, h)
                            queue_pool(4 + h, t, ml_sb, h)

                    def emit_pv_batch(e):
                        for t in range(CT):
                            lo, hi = spans[t]
                            if not (lo <= e < hi):
                                continue
                            nc.tensor.matmul(pvs[t][:],
                                             p_sb[:, e, t * P:(t + 1) * P],
                                             ev_s(e),
                                             start=(e == lo), stop=(e == hi - 1))
                            pop_pools(1)
                            nc.tensor.matmul(rs[:, t:t + 1],
                                             p_sb[:, e, t * P:(t + 1) * P],
                                             ones_col[:], start=first_rs[0],
                                             stop=(e == hi - 1),
                                             skip_group_check=True)
                            first_rs[0] = False
                            pop_pools(1)
                        for t in range(CT):
                            lo, hi = spans[t]
                            if e == hi - 1:
                                emit_complete(t)

                    # ---------- scores -> exp -> PV e-loop (sw-pipelined) ----
                    e_prev = None
                    for e in range(ET):
                        w0, w1 = windows[e]
                        s_ps = psS.tile([P, NHID], f32, tag="s",
                                        name=f"s{e}_{rep}")
                        nc.tensor.matmul(s_ps[:, :w1 - w0],
                                         wea_sb[:, e * P:(e + 1) * P],
                                         wca_sb[:, w0:w1], start=True, stop=True)
                        nc.scalar.activation(p_sb[:, e, w0:w1],
                                             s_ps[:, :w1 - w0], AF.Exp,
                                             bias=exp_bias[:])
                        if e_prev is not None:
                            emit_pv_batch(e_prev)
                        e_prev = e
                    emit_pv_batch(e_prev)

                    # ---------- tail: drain pools, copy segX^T, final matmul --
                    # fins split into two 256-col halves so the first half's
                    # output copy + DMA overlap the second half's matmuls
                    pop_pools(len(pend))
                    H = NHID // 2

                    def wab_cols(j, c0, c1):
                        return mega[:, o_wab + j * NHID + c0:
                                    o_wab + j * NHID + c1]

                    finA = psS.tile([P, NHID], f32, tag="s", name=f"fA_{rep}")
                    for j in range(KD):
                        nc.scalar.copy(segXT[:, j, :],
                                       bankA[:, j * PROJ:(j + 1) * PROJ])
                        nc.tensor.matmul(finA[:NG, :H], segXT[:, j, :],
                                         wab_cols(j, 0, H), start=(j == 0),
                                         stop=(j == KD - 1))
                    nc.scalar.copy(seg_sb[:, :H], finA[:NG, :H])
                    nc.sync.dma_start(seg_d[:, :H], seg_sb[:, :H])

                    finB = psS.tile([P, NHID], f32, tag="s", name=f"fB_{rep}")
                    for j in range(KD):
                        nc.tensor.matmul(finB[:NG, :H], segXT[:, j, :],
                                         wab_cols(j, H, NHID), start=(j == 0),
                                         stop=(j == KD - 1))
                    nc.scalar.copy(seg_sb[:, H:], finB[:NG, :H])
                    nc.sync.dma_start(seg_d[:, H:], seg_sb[:, H:])
    if split_waits:
        _split_excess_waits(nc)
    return nc


def make_in_maps(inputs: dict):
    """Host-side sharding: sort claims+evidence by graph, fp32 projections,
    pre-gather x rows (bf16) into per-core SBUF layouts, and compute the
    block-sparse envelope structure shared by all cores (SPMD)."""
    batch = np.asarray(inputs["batch"]).astype(np.int64)
    ci = np.asarray(inputs["claim_index"]).astype(np.int64)
    ei = np.asarray(inputs["evidence_index"]).astype(np.int64)
    x = np.asarray(inputs["x"], dtype=np.float32)
    cb = batch[ci]
    eb = batch[ei]
    counts = np.bincount(cb, minlength=NG).astype(np.float32)
    ba = np.asarray(inputs["ba"], dtype=np.float32).reshape(NHID)

    order_c = np.argsort(cb, kind="stable")
    cb_s = cb[order_c]
    order_e = np.argsort(eb, kind="stable")
    eb_s = eb[order_e]

    xc = x[ci[order_c]]             # [4096, 512] f32 sorted claims
    xe = x[ei[order_e]]             # [8192, 512] f32 sorted evidence
    ev_starts = np.searchsorted(eb_s, np.arange(NG + 1))

    # per-core contiguous evidence spans
    raw_spans = []
    for c in range(N_CORES):
        g_lo = int(cb_s[c * NC_LOC])
        g_hi = int(cb_s[(c + 1) * NC_LOC - 1])
        lo, hi = int(ev_starts[g_lo]), int(ev_starts[g_hi + 1])
        raw_spans.append((lo, hi))
    ne_loc = max(512, -(-max(hi - lo for lo, hi in raw_spans) // P) * P)
    ne_loc = min(ne_loc, NE)
    ET = ne_loc // P

    Wc = np.asarray(inputs["Wc"], dtype=np.float32)
    We = np.asarray(inputs["We"], dtype=np.float32)
    bc = np.asarray(inputs["bc"], dtype=np.float32).reshape(PROJ)
    be = np.asarray(inputs["be"], dtype=np.float32).reshape(PROJ)
    Wa = np.asarray(inputs["Wa"], dtype=np.float32)
    W1 = Wa[0:NHID] + Wa[2 * NHID:3 * NHID]
    W2 = Wa[NHID:2 * NHID] - Wa[2 * NHID:3 * NHID]
    W3 = Wa[3 * NHID:4 * NHID]
    # device handles the attention-dependent blocks (cn, ml); the claim
    # block's pooled contribution (Oh^T cl) @ W1' is computed on host
    wab = np.concatenate([W2, W3], axis=0).astype(nbf16)  # [1024, 512]

    # choose each core's evidence-window start (within its slack) to minimize
    # the cross-core envelope of per-claim-tile e-tile spans (coordinate
    # descent; spans quantize at 128-row tiles so candidates step by 32)
    def tile_spans(c, start):
        cb_c = cb_s[c * NC_LOC:(c + 1) * NC_LOC]
        out = []
        for t in range(CT):
            gmin = int(cb_c[t * P])
            gmax = int(cb_c[(t + 1) * P - 1])
            r0 = int(ev_starts[gmin]) - start
            r1 = int(ev_starts[gmax + 1]) - start
            out.append((r0 // P, -(-r1 // P)))
        return out

    # window start may be anywhere in [hi - ne_loc, lo]; rows past the end of
    # the global evidence array are padded with a sentinel graph (masked out),
    # so cores near the end are not force-slid into earlier graphs
    start_rng = []
    for c in range(N_CORES):
        lo, hi = raw_spans[c]
        start_rng.append((max(0, hi - ne_loc), lo))
    starts = [s_hi for (s_lo, s_hi) in start_rng]

    def env_cost(st):
        allsp = [tile_spans(c, st[c]) for c in range(N_CORES)]
        cost = 0
        for t in range(CT):
            cost += max(s[t][1] for s in allsp) - min(s[t][0] for s in allsp)
        return cost

    for _ in range(3):
        for c in range(N_CORES):
            s_lo, s_hi = start_rng[c]
            best = (env_cost(starts), starts[c])
            for cand in range(s_lo, s_hi + 1, 32):
                starts[c] = cand
                cc = env_cost(starts)
                if cc < best[0]:
                    best = (cc, cand)
            starts[c] = best[1]

    g_ids = np.arange(NG)
    in_maps = []
    env_spans = [[ET, 0] for _ in range(CT)]
    host_cl = np.zeros((NG, NHID), np.float64)
    for c in range(N_CORES):
        lo = starts[c]
        end = min(lo + ne_loc, NE)
        pad = lo + ne_loc - end
        xe_c = xe[lo:end]                          # [<=ne_loc, 512] f32
        eb_c = eb_s[lo:end]
        if pad:
            xe_c = np.concatenate([xe_c, np.zeros((pad, NHID), np.float32)], 0)
            eb_c = np.concatenate([eb_c, np.full(pad, 9999, eb_c.dtype)], 0)
        xc_c = xc[c * NC_LOC:(c + 1) * NC_LOC]     # [512, 512] f32
        cb_c = cb_s[c * NC_LOC:(c + 1) * NC_LOC]

        wc = (xc_c @ Wc + bc).T                    # [64, 512] f32
        we = (xe_c @ We + be).T                    # [64, ne_loc] f32
        oh = (cb_c[:, None] == g_ids[None, :])     # [512, 64]
        host_cl += (oh.T.astype(np.float64) @ xc_c) @ W1

        wca = np.concatenate(
            [wc, MAG * (cb_c[None, :] == g_ids[:PROJ, None])], 0).astype(nbf16)
        wea = np.concatenate(
            [we, MAG * (eb_c[None, :] == g_ids[:PROJ, None])], 0).astype(nbf16)
        ev = xe_c.astype(nbf16).reshape(ET, P, NHID).transpose(1, 0, 2)
        cl = xc_c.astype(nbf16).reshape(CT, P, NHID).transpose(1, 0, 2)
        ohs = oh.reshape(CT, P, NG).transpose(1, 0, 2).astype(nbf16)
        wabt = wab.reshape(8, P, NHID).transpose(1, 0, 2)
        mega = np.concatenate(
            [wca, wea,
             ev.reshape(P, -1), cl.reshape(P, -1), ohs.reshape(P, -1),
             wabt.reshape(P, -1)], axis=1)
        in_maps.append({"mega": np.ascontiguousarray(mega)})

        # per-core per-c-tile evidence e-tile spans -> envelope
        for t in range(CT):
            gmin = int(cb_c[t * P])
            gmax = int(cb_c[(t + 1) * P - 1])
            r0 = int(np.searchsorted(eb_c, gmin))
            r1 = int(np.searchsorted(eb_c, gmax, side="right"))
            assert r1 > r0, "claim tile with no evidence in its graphs"
            env_spans[t][0] = min(env_spans[t][0], r0 // P)
            env_spans[t][1] = max(env_spans[t][1], -(-r1 // P))

    # enforce monotone lo/hi (expand-only) so claim windows are contiguous
    for t in range(CT - 2, -1, -1):
        env_spans[t][0] = min(env_spans[t][0], env_spans[t + 1][0])
    for t in range(1, CT):
        env_spans[t][1] = max(env_spans[t][1], env_spans[t - 1][1])
    spans = [(lo, hi) for lo, hi in env_spans]

    windows = []
    for e in range(ET):
        ts = [t for t in range(CT) if spans[t][0] <= e < spans[t][1]]
        assert ts, f"e-tile {e} covered by no claim tile"
        assert ts == list(range(min(ts), max(ts) + 1))
        windows.append((min(ts) * P, (max(ts) + 1) * P))

    struct = {"ne_loc": ne_loc, "et": ET, "spans": spans, "windows": windows,
              "host_cl": host_cl}
    return in_maps, counts, ba, struct


def postprocess(results: list, counts: np.ndarray, ba: np.ndarray,
                struct: dict) -> np.ndarray:
    seg = struct["host_cl"].copy()
    for c in range(N_CORES):
        seg += results[c]["seg"].astype(np.float64)
    # segment_mean(a + ba) = segment_mean(a) + ba, except empty graphs stay 0
    out = seg / np.maximum(counts, 1.0)[:, None] + (counts > 0)[:, None] * ba[None, :]
    return out.astype(np.float32)


def kernel(**inputs) -> np.ndarray:
    in_maps, counts, ba, struct = make_in_maps(inputs)
    nc = build_nc(struct)
    res = run_bass_kernel_spmd(nc, in_maps, list(range(N_CORES)))
    return postprocess(res.results, counts, ba, struct)
